# revision 1
# baseline (speedup 1.0000x reference)
"""Bass/Tile TRN2 kernel for nn_BayesHead (projected single-head attention,
near-causal mask tril(diag=1), double 1/sqrt(64) scaling).

Strategy (8 NeuronCores, pure data-parallel SPMD — no collectives):
  - core j handles batch b = j//2 with key-parity p = j%2.
  - Each core projects ALL 4096 queries of its batch, and its HALF of the
    keys/values (interleaved 128-row blocks: global block g = 2*sigma + p).
  - Flash-style partial softmax without max-subtraction (scores are in
    [-1,1] after the 1/64 scaling, so exp is safe): each core produces
    O_p[h, t] = sum_{s in its keys, s <= t+1} exp(S) * V[s, h] plus a
    denominator row (ones-column trick).  The host sums the two partials
    per batch and normalizes.

v2 scheduling (vs the 109us baseline):
  - DRAM inputs pre-laid-out as [128, ct, t] so one dma_start per 512-col
    chunk lands contiguously; chunks issued in exact consumption order so
    all 16 DMA engines run from t~0 and compute starts at ~4us.
  - Mask tensors built on the (otherwise idle) GPSIMD engine so the DVE
    stream never blocks early PSUM-evacuation copies.
  - Projections interleaved with attention tiles so the PE stays
    continuously busy (HAM ramps to 2.4 GHz) while the ACT engine chews
    the exp stream in parallel.
  - The last s-tile of each query tile is >99% masked (only its first key
    is visible, to the last query): scores/exp/mask/PV are trimmed to the
    final 128 columns there.
"""

import numpy as np
from contextlib import ExitStack

import concourse.bass as bass
import concourse.mybir as mybir
import concourse.tile as tile
from concourse import bacc
from concourse.bass import ts
from concourse.bass_utils import run_bass_kernel_spmd

B, T, C, H = 4, 4096, 1024, 64
NCORES = 8
TQ = 512                       # query-tile width
NQT = T // TQ                  # 8 query tiles
NSB = (T // 2) // 128          # 16 local key tiles (128 rows each)
NCT = C // 128                 # 8 contraction tiles
TH = T // 2
# s-tile capacity per query tile (identical for both parities; covers causal
# reach ceil((4i+5)/2), capped at the 16 local tiles)
CAPS = [min(NSB, 2 * i + 3) for i in range(NQT)]
MASK_FROM = [2 * i for i in range(NQT)]  # sigma >= 2i may cross the diagonal
MASKED = [(i, s) for i in range(NQT) for s in range(MASK_FROM[i], CAPS[i])]
M_IDX = {k: m for m, k in enumerate(MASKED)}
N_MASKED = len(MASKED)
W0 = 384                       # live-column window start for singleton s-tiles
FP = mybir.dt.float16
F32 = mybir.dt.float32
SCALE = 1.0 / H                # (H**-0.5) applied twice


def build_bass():
    nc = bacc.Bacc("TRN2", target_bir_lowering=False, num_devices=NCORES)
    # DRAM layouts are pre-transposed on host and chunk-major:
    # x[p, chunk, ct, col] = x.T[128*ct+p, 512*chunk+col], so each 512-col
    # chunk DMA moves 8KB contiguous per partition (128 fat descriptors)
    qT = nc.declare_dram_parameter("qT", [128, NQT, NCT, 512], FP, isOutput=False)
    kT = nc.declare_dram_parameter("kT", [128, NQT // 2, NCT, 512], FP, isOutput=False)
    vT = nc.declare_dram_parameter("vT", [128, NQT // 2, NCT, 512], FP, isOutput=False)
    wq = nc.declare_dram_parameter("wq", [128, NCT, H], FP, isOutput=False)
    wk = nc.declare_dram_parameter("wk", [128, NCT, H], FP, isOutput=False)
    wv = nc.declare_dram_parameter("wv", [128, NCT, H], FP, isOutput=False)
    iota = nc.declare_dram_parameter("iota", [128, TQ], FP, isOutput=False)
    thr = nc.declare_dram_parameter("thr", [128, N_MASKED], F32, isOutput=False)
    ident = nc.declare_dram_parameter("ident", [64, 64], FP, isOutput=False)
    out = nc.declare_dram_parameter("out", [H + 1, T], F32, isOutput=True)

    with ExitStack() as ctx:
        tc = ctx.enter_context(tile.TileContext(nc))
        singles = ctx.enter_context(tc.tile_pool(name="singles", bufs=1))
        pt_pool = ctx.enter_context(tc.tile_pool(name="pt", bufs=6))
        outsb_pool = ctx.enter_context(tc.tile_pool(name="outsb", bufs=6))
        stage_pool = ctx.enter_context(tc.tile_pool(name="stage", bufs=2))
        psum_s = ctx.enter_context(tc.tile_pool(name="psum_s", bufs=3, space="PSUM"))
        psum_o = ctx.enter_context(tc.tile_pool(name="psum_o", bufs=2, space="PSUM"))

        # SBUF-resident tiles
        iota_sb = singles.tile([128, TQ], FP)
        thr_sb = singles.tile([128, N_MASKED], F32)
        wq_sb = singles.tile([128, NCT, H], FP)
        wk_sb = singles.tile([128, NCT, H], FP)
        wv_sb = singles.tile([128, NCT, H], FP)
        id_sb = singles.tile([64, 64], FP)
        q_sb = singles.tile([128, NQT, NCT, 512], FP)
        k_sb = singles.tile([128, NQT // 2, NCT, 512], FP)
        v_sb = singles.tile([128, NQT // 2, NCT, 512], FP)

        qp_sb = singles.tile([128, T], FP)        # Q^T [h, t], dup on parts 64-127
        kp_sb = singles.tile([128, TH], FP)       # K^T [h, s], dup on parts 64-127
        va_sb = singles.tile([128, NSB, H + 1], FP)  # V rows [s, h] + ones col
        masks_sb = singles.tile([128, N_MASKED, TQ], FP)

        # ---- DMA issue stream (sync engine), in consumption order ----
        def dq(c):
            nc.sync.dma_start(out=q_sb[:, c, :, :], in_=qT[:, c, :, :])

        def dk(c):
            nc.sync.dma_start(out=k_sb[:, c, :, :], in_=kT[:, c, :, :])

        def dv(c):
            nc.sync.dma_start(out=v_sb[:, c, :, :], in_=vT[:, c, :, :])

        nc.sync.dma_start(out=iota_sb, in_=iota[:, :])
        nc.sync.dma_start(out=thr_sb, in_=thr[:, :])
        nc.sync.dma_start(out=wk_sb, in_=wk[:, :, :])
        dk(0)
        nc.sync.dma_start(out=wq_sb, in_=wq[:, :, :])
        dq(0)
        nc.sync.dma_start(out=wv_sb, in_=wv[:, :, :])
        dv(0)
        nc.sync.dma_start(out=id_sb, in_=ident[:, :])
        dq(1); dk(1); dv(1)
        dq(2); dk(2); dv(2)
        dq(3)
        dq(4); dk(3); dv(3)
        dq(5); dq(6); dq(7)

        # ones column for the softmax denominator
        nc.vector.memset(va_sb[:, :, H:H + 1], 1.0)



        def build_masks(i):
            # DVE mask builds, emitted just before attention(i) so they never
            # stall the early PSUM-evacuation copies in the DVE stream
            for s in range(MASK_FROM[i], CAPS[i]):
                m = M_IDX[(i, s)]
                sub = slice(W0, TQ) if (i < 7 and s == CAPS[i] - 1) else slice(0, TQ)
                nc.vector.tensor_scalar(
                    masks_sb[:, m, sub], iota_sb[:, sub], thr_sb[:, m:m + 1],
                    None, mybir.AluOpType.is_ge)

        def q_proj(tq):
            pq = psum_s.tile([128, 512], F32, tag="ps")
            for ct in range(NCT):
                nc.tensor.matmul(pq[0:64, :], wq_sb[:, ct, :],
                                 q_sb[:, tq, ct, :], tile_position=(0, 0),
                                 start=(ct == 0), stop=(ct == NCT - 1))
                nc.tensor.matmul(pq[64:128, :], wq_sb[:, ct, :],
                                 q_sb[:, tq, ct, :], tile_position=(0, 64),
                                 start=(ct == 0), stop=(ct == NCT - 1),
                                 skip_group_check=True)
            nc.vector.tensor_copy(qp_sb[:, ts(tq, 512)], pq)

        def k_proj(c4):
            pk = psum_s.tile([128, 512], F32, tag="ps")
            for ct in range(NCT):
                nc.tensor.matmul(pk[0:64, :], wk_sb[:, ct, :],
                                 k_sb[:, c4, ct, :], tile_position=(0, 0),
                                 start=(ct == 0), stop=(ct == NCT - 1))
                nc.tensor.matmul(pk[64:128, :], wk_sb[:, ct, :],
                                 k_sb[:, c4, ct, :], tile_position=(0, 64),
                                 start=(ct == 0), stop=(ct == NCT - 1),
                                 skip_group_check=True)
            nc.vector.tensor_copy(kp_sb[:, ts(c4, 512)], pk)

        def v_proj(c4):
            pv = psum_s.tile([64, 512], F32, tag="ps")
            for ct in range(NCT):
                nc.tensor.matmul(pv, wv_sb[:, ct, :], v_sb[:, c4, ct, :],
                                 start=(ct == 0), stop=(ct == NCT - 1))
            vt_stage = stage_pool.tile([64, 512], FP)
            nc.vector.tensor_copy(vt_stage, pv)
            for j in range(4):
                sig = c4 * 4 + j
                ptr = psum_o.tile([128, H], FP, tag="oacc")
                nc.tensor.transpose(ptr, vt_stage[:, ts(j, 128)], id_sb)
                nc.vector.tensor_copy(va_sb[:, sig, 0:H], ptr)

        def attention(i):
            cap = CAPS[i]
            po = psum_o.tile([H + 1, 512], F32, tag="oacc")
            # group list: leading full pairs, then (i<7) the nearly-dead
            # singleton (trimmed to cols [W0,512)), then the diagonal pair
            # (2i, 2i+1) last so the accumulation stop lands on a full-width
            # matmul.
            groups = []
            lead = cap if i == 7 else 2 * i
            for g0 in range(0, lead, 2):
                groups.append(("pair", g0))
            if i == 0:
                # the first PV writing po must be full width (PSUM zero-region
                # start semantics), so the trimmed singleton goes last
                groups = [("pair", 0), ("single", 2)]
            elif i < 7:
                groups.append(("single", cap - 1))
                groups.append(("pair", 2 * i))
            def emit_s(kind, g0):
                # scores matmuls + exp + mask for one group; returns pt handle
                if kind == "pair":
                    ps = psum_s.tile([128, 1024], F32, tag="ps")
                    for g in (0, 1):
                        sig = g0 + g
                        nc.tensor.matmul(ps[:, ts(g, 512)],
                                         kp_sb[ts(g, 64), ts(sig, 128)],
                                         qp_sb[ts(g, 64), ts(i, 512)],
                                         tile_position=(64 * g, 0),
                                         start=True, stop=True)
                    pt = pt_pool.tile([128, 1024], FP)
                    nc.scalar.activation(pt, ps,
                                         mybir.ActivationFunctionType.Exp,
                                         scale=SCALE)
                    for g in (0, 1):
                        sig = g0 + g
                        if sig >= MASK_FROM[i]:
                            m = M_IDX[(i, sig)]
                            nc.vector.tensor_mul(pt[:, ts(g, 512)],
                                                 pt[:, ts(g, 512)],
                                                 masks_sb[:, m, :])
                else:  # singleton: full-width scores (PSUM zero-region rule),
                    # but exp/mask/PV trimmed to the live cols [W0:512)
                    sig = g0
                    m = M_IDX[(i, sig)]
                    ps = psum_s.tile([128, 512], F32, tag="ps")
                    nc.tensor.matmul(ps,
                                     kp_sb[0:64, ts(sig, 128)],
                                     qp_sb[0:64, ts(i, 512)],
                                     tile_position=(0, 0), start=True, stop=True)
                    pt = pt_pool.tile([128, 512], FP)
                    nc.scalar.activation(pt[:, W0:512], ps[:, W0:512],
                                         mybir.ActivationFunctionType.Exp,
                                         scale=SCALE)
                    nc.vector.tensor_mul(pt[:, W0:512], pt[:, W0:512],
                                         masks_sb[:, m, W0:512])
                return pt

            state = {"first": True}

            def emit_pv(kind, g0, pt, last_grp):
                if kind == "pair":
                    for g in (0, 1):
                        sig = g0 + g
                        nc.tensor.matmul(po, va_sb[:, sig, :], pt[:, ts(g, 512)],
                                         start=state["first"],
                                         stop=(last_grp and g == 1))
                        state["first"] = False
                else:
                    nc.tensor.matmul(po[:, W0:512], va_sb[:, g0, :],
                                     pt[:, W0:512], start=state["first"],
                                     stop=last_grp)
                    state["first"] = False

            # software pipeline: PV stream lags the scores stream by 2 groups
            # so the PE never stalls on the exp+mask latency
            LAG = 2
            pend = []
            for gi, (kind, g0) in enumerate(groups):
                pt = emit_s(kind, g0)
                pend.append((kind, g0, pt))
                if gi >= LAG:
                    k_, g_, pt_ = pend.pop(0)
                    emit_pv(k_, g_, pt_, last_grp=False)
            for j, (k_, g_, pt_) in enumerate(pend):
                emit_pv(k_, g_, pt_, last_grp=(j == len(pend) - 1))
            osb = outsb_pool.tile([H + 1, 512], F32)
            nc.vector.tensor_copy(osb, po)
            nc.sync.dma_start(out=out[:, ts(i, 512)], in_=osb)

        # ---- compute schedule: keep PE fed, ACT streaming, DMA-aligned ----
        # attention(i) needs q_proj(i) and k/v chunks 0..(CAPS[i]-1)//4
        k_proj(0); q_proj(0); build_masks(0); v_proj(0)
        attention(0)
        q_proj(1); build_masks(1); k_proj(1); v_proj(1)
        attention(1)
        q_proj(2); build_masks(2)
        attention(2)
        k_proj(2); v_proj(2); q_proj(3); build_masks(3)
        attention(3)
        q_proj(4); build_masks(4)
        attention(4)
        k_proj(3); v_proj(3); q_proj(5); build_masks(5)
        attention(5)
        q_proj(6); build_masks(6)
        attention(6)
        q_proj(7); build_masks(7)
        attention(7)

    nc.compile()
    return nc


_NC = None


def _get_nc():
    global _NC
    if _NC is None:
        _NC = build_bass()
    return _NC


def _prep_core_inputs(q, k, v, Wq, Wk, Wv):
    f2 = np.float16

    def wprep(W):
        # SBUF layout [p, ct, h] = W.T[ct*128+p, h]
        return np.ascontiguousarray(W.T.reshape(NCT, 128, H).transpose(1, 0, 2)).astype(f2)

    def xprep(x):
        # [p, chunk, ct, col] = x.T[128*ct+p, 512*chunk+col]
        xt = x.T.astype(f2)                       # [C, T']
        nch = xt.shape[1] // 512
        return np.ascontiguousarray(
            xt.reshape(NCT, 128, nch, 512).transpose(1, 2, 0, 3))

    wq_h, wk_h, wv_h = wprep(Wq), wprep(Wk), wprep(Wv)
    iota_h = np.ascontiguousarray(
        np.broadcast_to(np.arange(TQ, dtype=np.float32), (128, TQ))).astype(f2)
    ident_h = np.eye(64, dtype=f2)

    r = np.arange(128)
    in_maps = []
    for j in range(NCORES):
        b, p = j // 2, j % 2
        rows = (np.arange(TH) // 128) * 256 + p * 128 + (np.arange(TH) % 128)
        qT_h = xprep(q[b])
        kT_h = xprep(k[b][rows])
        vT_h = xprep(v[b][rows])
        thr_h = np.empty((128, N_MASKED), np.float32)
        for m, (i, s) in enumerate(MASKED):
            t = 128 * (2 * s + p) + r - TQ * i - 1
            thr_h[:, m] = np.clip(t, -1024, 1024).astype(np.float32)
        in_maps.append({
            "qT": qT_h, "kT": kT_h, "vT": vT_h,
            "wq": wq_h, "wk": wk_h, "wv": wv_h,
            "iota": iota_h, "thr": thr_h, "ident": ident_h,
        })
    return in_maps


def _run(inputs, trace=False, trace_kwargs=None):
    nc = _get_nc()
    in_maps = _prep_core_inputs(
        inputs["q"], inputs["k"], inputs["v"],
        inputs["Wq"], inputs["Wk"], inputs["Wv"])
    res = run_bass_kernel_spmd(nc, in_maps, list(range(NCORES)), trace=trace,
                               **(trace_kwargs or {}))
    outs = [res.results[j]["out"] for j in range(NCORES)]
    y = np.empty((B, T, H), np.float32)
    for b in range(B):
        s = outs[2 * b] + outs[2 * b + 1]      # [H+1, T]
        y[b] = (s[:H] / s[H:H + 1]).T
    return y, res


def kernel(q, k, v, Wq, Wk, Wv):
    y, _ = _run({"q": np.asarray(q), "k": np.asarray(k), "v": np.asarray(v),
                 "Wq": np.asarray(Wq), "Wk": np.asarray(Wk), "Wv": np.asarray(Wv)})
    return y

